# revision 1
# baseline (speedup 1.0000x reference)
"""Causal multi-head attention (B=4, S=2048, D=1024, H=16) on 8 Trainium2 cores.

Sharding: core c handles batch b = c//2 and head-half hh = c%2 (8 heads, 512
head-dims). QKV/out projections are tensor-parallel over the head dim;
attention is embarrassingly parallel over (b, head). The out-projection
partial products (rank-512 each) are summed pairwise on the host along with
the output bias.

On-device layout is fully transposed (d_model / head-dim on partitions,
sequence on the free axis) so every matmul contracts over the partition dim
with no on-chip transposes:
  Q^T = (Wq*scale)-tiles.T @ x^T      K^T likewise     V natural = x^T.T @ Wv
  S^T[k,q] = K_h @ Q_h^T              exp on ScalarE (no max subtraction:
                                      |scores| <~ 5, exp is safe in fp32)
  [O^T; l..l] = [V_h | 1s].T @ expS^T (64 ones columns in the stationary make
                                      PV emit the softmax denominator
                                      replicated on partitions 64..127)
  O_norm^T = O^T * (1/l)              Y^T-partial = Wo-tiles.T @ O_cat^T
Causality: fully-masked key-blocks are skipped; diagonal blocks compute only
the valid column range and apply one 128x128 triangular mask multiply.
Head pairs (even/odd) interleave so K=64 score matmuls pack into disjoint
PE row-groups and run concurrently.
"""

import numpy as np
import ml_dtypes

B, S, D = 4, 2048, 1024
H = 16
HH = 8          # heads per core
DK = 64
HD = 512        # head dims per core
N_CORES = 8
SCALE = DK ** -0.5
PB = 128        # partition block
QB = 512        # query block (matmul free dim)
NQB = S // QB   # 4
NKB = S // PB   # 16
KD = D // PB    # 8
KO = HD // PB   # 4

_COMPILED = None
LAST_RESULTS = None


def _build():
    from contextlib import ExitStack
    import concourse.bass as bass
    import concourse.tile as tile
    from concourse import bacc, mybir

    BF16 = mybir.dt.bfloat16
    F32 = mybir.dt.float32
    AF = mybir.ActivationFunctionType

    nc = bacc.Bacc("TRN2", target_bir_lowering=False, debug=False,
                   num_devices=N_CORES)

    xT_d = nc.dram_tensor("xT", [D, S], BF16, kind="ExternalInput")
    wq_d = nc.dram_tensor("wq", [D, HD], BF16, kind="ExternalInput")
    wk_d = nc.dram_tensor("wk", [D, HD], BF16, kind="ExternalInput")
    wv_d = nc.dram_tensor("wv", [D, HD], BF16, kind="ExternalInput")
    wo_d = nc.dram_tensor("wo", [HD, D], BF16, kind="ExternalInput")
    bq_d = nc.dram_tensor("bq", [PB, KO], F32, kind="ExternalInput")
    bk_d = nc.dram_tensor("bk", [PB, KO], F32, kind="ExternalInput")
    bvb_d = nc.dram_tensor("bvb", [PB, HD], BF16, kind="ExternalInput")
    mk_d = nc.dram_tensor("mk", [PB, 2 * PB], BF16, kind="ExternalInput")
    yT_d = nc.dram_tensor("yT", [D, S], F32, kind="ExternalOutput")

    with tile.TileContext(nc) as tc, ExitStack() as ctx:
        persist = ctx.enter_context(tc.tile_pool(name="persist", bufs=1))
        work = ctx.enter_context(tc.tile_pool(name="work", bufs=6))
        nrm = ctx.enter_context(tc.tile_pool(name="nrm", bufs=4))
        psA = ctx.enter_context(tc.tile_pool(name="psA", bufs=3, space="PSUM"))
        psO = ctx.enter_context(tc.tile_pool(name="psO", bufs=2, space="PSUM"))

        xT = [persist.tile([PB, S], BF16, name=f"xT{k}") for k in range(KD)]
        wq = [persist.tile([PB, HD], BF16, name=f"wq{k}") for k in range(KD)]
        wk = [persist.tile([PB, HD], BF16, name=f"wk{k}") for k in range(KD)]
        wv = [persist.tile([PB, HD], BF16, name=f"wv{k}") for k in range(KD)]
        wo = [persist.tile([PB, D], BF16, name=f"wo{k}") for k in range(KO)]
        bq = persist.tile([PB, KO], F32)
        bk = persist.tile([PB, KO], F32)
        bvb = persist.tile([PB, HD], BF16)
        mk = persist.tile([PB, 2, PB], BF16)
        qT = [persist.tile([PB, S], BF16, name=f"qT{k}") for k in range(KO)]
        kT = [persist.tile([PB, S], BF16, name=f"kT{k}") for k in range(KO)]
        # per key-block: 8 heads x (64 ones columns + 64 V columns).
        # Ones first so the PV matmul puts the softmax denominator on
        # partitions 0..63 (reciprocal_approx_fast requires base partition 0).
        v = [persist.tile([PB, HH, 2 * DK], BF16, name=f"v{k}") for k in range(NKB)]
        onorm = [persist.tile([PB, S], BF16, name=f"onorm{k}") for k in range(KO)]

        # x and Wv first (V projection gates attention); two DMA queues
        for k in range(KD):
            nc.sync.dma_start(xT[k][:], xT_d[k * PB:(k + 1) * PB, :])
            nc.gpsimd.dma_start(wv[k][:], wv_d[k * PB:(k + 1) * PB, :])
        for k in range(KD):
            nc.gpsimd.dma_start(wq[k][:], wq_d[k * PB:(k + 1) * PB, :])
            nc.sync.dma_start(wk[k][:], wk_d[k * PB:(k + 1) * PB, :])
        for k2 in range(KO):
            nc.gpsimd.dma_start(wo[k2][:], wo_d[k2 * PB:(k2 + 1) * PB, :])
        nc.sync.dma_start(bq[:], bq_d[:])
        nc.sync.dma_start(bk[:], bk_d[:])
        nc.gpsimd.dma_start(bvb[:], bvb_d[:])
        nc.sync.dma_start(mk[:], mk_d[:].rearrange("p (j c) -> p j c", j=2))
        for kb in range(NKB):
            nc.vector.memset(v[kb][:, :, 0:DK], 1.0)

        # V projection first (attention needs it for every head); two key
        # blocks share one 2-bank accumulator tile
        for kb2 in range(NKB // 2):
            accv = psA.tile([PB, 2, QB], F32, tag="acc")
            for j in range(2):
                kb = 2 * kb2 + j
                for k in range(KD):
                    nc.tensor.matmul(accv[:, j, :],
                                     xT[k][:, kb * PB:(kb + 1) * PB],
                                     wv[k][:], start=(k == 0),
                                     stop=(k == KD - 1))
            for j in range(2):
                kb = 2 * kb2 + j
                nc.vector.tensor_add(
                    v[kb][:, :, DK:2 * DK],
                    accv[:, j, :].rearrange("p (h d) -> p h d", h=HH),
                    bvb[:].rearrange("p (h d) -> p h d", h=HH))

        def qk_proj(m, qb):
            qs = slice(qb * QB, (qb + 1) * QB)
            acc2 = psA.tile([PB, 2, QB], F32, tag="acc")
            for k in range(KD):
                nc.tensor.matmul(acc2[:, 0, :], wq[k][:, m * PB:(m + 1) * PB],
                                 xT[k][:, qs], start=(k == 0),
                                 stop=(k == KD - 1))
            for k in range(KD):
                nc.tensor.matmul(acc2[:, 1, :], wk[k][:, m * PB:(m + 1) * PB],
                                 xT[k][:, qs], start=(k == 0),
                                 stop=(k == KD - 1))
            nc.vector.tensor_scalar_add(qT[m][:, qs], acc2[:, 0, :],
                                        bq[:, m:m + 1])
            nc.vector.tensor_scalar_add(kT[m][:, qs], acc2[:, 1, :],
                                        bk[:, m:m + 1])

        def attention(hp, qb):
            # head pair 2*hp (rows 0:64) + 2*hp+1 (rows 64:128), query block qb
            m = hp
            qs = slice(qb * QB, (qb + 1) * QB)
            nkb = 4 * qb + 4
            o_accs = [psO.tile([PB, QB], F32, tag="oacc", name=f"oacc{i}")
                      for i in range(2)]
            for kb in range(nkb):
                t = kb - 4 * qb
                c0 = 0 if t < 0 else 128 * t
                cs = slice(qb * QB + c0, (qb + 1) * QB)
                s_ps = psA.tile([PB, 2, QB], F32, tag="acc")
                for i, rb in enumerate((0, DK)):
                    nc.tensor.matmul(
                        s_ps[:, i, c0:QB],
                        kT[m][rb:rb + DK, kb * PB:(kb + 1) * PB],
                        qT[m][rb:rb + DK, cs], start=True, stop=True)
                e_sb = work.tile([PB, 2, QB], BF16, tag="exp")
                nc.scalar.activation(e_sb[:, :, c0:QB], s_ps[:, :, c0:QB],
                                     AF.Exp)
                if t >= 0:
                    nc.vector.tensor_mul(e_sb[:, :, c0:c0 + PB],
                                         e_sb[:, :, c0:c0 + PB], mk[:])
                for i in range(2):
                    h = 2 * hp + i
                    nc.tensor.matmul(o_accs[i][:, c0:QB],
                                     v[kb][:, h, :], e_sb[:, i, c0:QB],
                                     start=(kb == 0), stop=(kb == nkb - 1),
                                     skip_group_check=True)
            for i, rb in enumerate((0, DK)):
                r_sb = nrm.tile([DK, QB], F32, tag="r")
                nc.vector.reciprocal_approx_fast(r_sb[:], o_accs[i][0:DK, :])
                nc.vector.tensor_mul(onorm[m][rb:rb + DK, qs],
                                     o_accs[i][DK:2 * DK, :], r_sb[:])

        # Interleave projection chunks between attention blocks so the PE has
        # independent work while ScalarE drains the exp backlog
        for qb in range(NQB):
            qk_proj(0, qb)
        for hp in range(KO):
            for qb in range(NQB):
                attention(hp, qb)
                if hp + 1 < KO:
                    qk_proj(hp + 1, qb)

        # Out projection partial: Y^T = sum_k2 Wo[k2].T @ O_cat^T[k2]
        for mo2 in range(KD // 2):
            for qb in range(NQB):
                qs = slice(qb * QB, (qb + 1) * QB)
                y_ps = psA.tile([PB, 2, QB], F32, tag="acc")
                for j in range(2):
                    mo = 2 * mo2 + j
                    for k2 in range(KO):
                        nc.tensor.matmul(y_ps[:, j, :],
                                         wo[k2][:, mo * PB:(mo + 1) * PB],
                                         onorm[k2][:, qs], start=(k2 == 0),
                                         stop=(k2 == KO - 1))
                y_sb = nrm.tile([PB, 2, QB], F32, tag="y")
                nc.vector.tensor_copy(y_sb[:], y_ps[:])
                nc.sync.dma_start(
                    yT_d[2 * mo2 * PB:(2 * mo2 + 2) * PB, qs]
                    .rearrange("(j p) q -> p j q", j=2), y_sb[:])

    nc.compile()
    return nc


def _get_compiled():
    global _COMPILED
    if _COMPILED is None:
        _COMPILED = _build()
    return _COMPILED


def _make_in_maps(x, Wq, bq, Wk, bk, Wv, bv, Wo):
    bf16 = ml_dtypes.bfloat16
    f32 = np.float32

    # inclusive lower-triangular mask for diagonal 128x128 blocks
    p_idx = np.arange(PB)[:, None]
    c_idx = np.arange(PB)[None, :]
    mk = np.tile((p_idx <= c_idx).astype(bf16), (1, 2))

    in_maps = []
    for c in range(N_CORES):
        b, hh = c // 2, c % 2
        cs = slice(hh * HD, (hh + 1) * HD)
        in_maps.append({
            "xT": np.ascontiguousarray(x[b].T).astype(bf16),
            "wq": (Wq[:, cs] * SCALE).astype(bf16),
            "wk": np.ascontiguousarray(Wk[:, cs]).astype(bf16),
            "wv": np.ascontiguousarray(Wv[:, cs]).astype(bf16),
            "wo": np.ascontiguousarray(Wo[cs, :]).astype(bf16),
            "bq": np.ascontiguousarray(
                (bq[cs] * SCALE).astype(f32).reshape(KO, PB).T),
            "bk": np.ascontiguousarray(
                bk[cs].astype(f32).reshape(KO, PB).T),
            "bvb": np.broadcast_to(bv[cs].astype(bf16), (PB, HD)).copy(),
            "mk": mk,
        })
    return in_maps


def _reference_fallback(x, mask, Wq, bq, Wk, bk, Wv, bv, Wo, bo):
    out = np.empty((B, S, D), dtype=np.float32)
    for b in range(B):
        q = (x[b] @ Wq + bq).reshape(S, H, DK).transpose(1, 0, 2)
        k = (x[b] @ Wk + bk).reshape(S, H, DK).transpose(1, 0, 2)
        vv = (x[b] @ Wv + bv).reshape(S, H, DK).transpose(1, 0, 2)
        o = np.empty((H, S, DK), dtype=np.float32)
        for hi in range(H):
            s = (q[hi] @ k[hi].T) * SCALE
            s = np.where(mask[b], -1e9, s)
            s = s - s.max(axis=-1, keepdims=True)
            e = np.exp(s)
            p = e / e.sum(axis=-1, keepdims=True)
            o[hi] = p @ vv[hi]
        out[b] = o.transpose(1, 0, 2).reshape(S, D) @ Wo + bo
    return out


def kernel(x, mask, Wq, bq, Wk, bk, Wv, bv, Wo, bo, **kwargs):
    global LAST_RESULTS
    import os

    x = np.asarray(x, dtype=np.float32)
    mask = np.asarray(mask)

    causal = np.triu(np.ones((S, S), dtype=bool), k=1)
    if not all(np.array_equal(mask[b], causal) for b in range(B)):
        return _reference_fallback(np.asarray(x), mask, np.asarray(Wq),
                                   np.asarray(bq), np.asarray(Wk),
                                   np.asarray(bk), np.asarray(Wv),
                                   np.asarray(bv), np.asarray(Wo),
                                   np.asarray(bo))

    from concourse.bass_utils import run_bass_kernel_spmd

    nc = _get_compiled()
    in_maps = _make_in_maps(x, np.asarray(Wq), np.asarray(bq), np.asarray(Wk),
                            np.asarray(bk), np.asarray(Wv), np.asarray(bv),
                            np.asarray(Wo))
    trace = bool(int(os.environ.get("KERNEL_PROFILE", "0")))
    res = run_bass_kernel_spmd(nc, in_maps, list(range(N_CORES)), trace=trace)
    LAST_RESULTS = res

    bo32 = np.asarray(bo, dtype=np.float32)
    out = np.empty((B, S, D), dtype=np.float32)
    for b in range(B):
        acc = res.results[2 * b]["yT"] + res.results[2 * b + 1]["yT"]
        out[b] = acc.T + bo32
    return out



# revision 2
# speedup vs baseline: 1.0149x; 1.0149x over previous
"""Causal multi-head attention (B=4, S=2048, D=1024, H=16) on 8 Trainium2 cores.

Sharding: core c handles batch b = c//2 and head-half hh = c%2 (8 heads, 512
head-dims). QKV/out projections are tensor-parallel over the head dim;
attention is embarrassingly parallel over (b, head). The out-projection
partial products (rank-512 each) are summed pairwise on the host along with
the output bias.

On-device layout is fully transposed (d_model / head-dim on partitions,
sequence on the free axis) so every matmul contracts over the partition dim
with no on-chip transposes:
  Q^T = (Wq*scale)-tiles.T @ x^T      K^T likewise     V natural = x^T.T @ Wv
  S^T[k,q] = K_h @ Q_h^T              exp on ScalarE (no max subtraction:
                                      |scores| <~ 5, exp is safe in fp32)
  [l..l; O^T] = [1s | V_h].T @ expS^T (64 ones columns in the stationary make
                                      PV emit the softmax denominator
                                      replicated on partitions 0..63)
  O_norm^T = O^T * (1/l)              Y^T-partial = Wo-tiles.T @ O_cat^T
Causality: fully-masked key-blocks are skipped; diagonal blocks compute only
the valid column range and apply one 128x128 triangular mask multiply.
Head pairs (even/odd) interleave so K=64 score matmuls pack into disjoint
PE row-groups and run concurrently.

Schedule notes (v2):
  - ScalarE exp demand inside an attention block exceeds the PE demand, so
    the PE needs independent projection work to chew on while exp drains.
    Scores, PV accumulators and projection accumulators live in separate
    PSUM pools (4+2+2 banks) so projection matmuls are never blocked on a
    score buffer held by ScalarE.
  - ~14 warm-up matmuls on a zeroed tile run at t=0 so the PE HAM clock
    gate lifts (1.2 -> 2.4 GHz) during the DMA lead-in instead of ~35us in;
    a few more zero-dep matmuls at the tail of the program act as idle
    filler wherever the scheduler finds a PE bubble.
  - Input DMA order: biases/mask first, then V/Q/K weights interleaved
    per k-block on one queue while x^T streams (split in column halves)
    on the other; Wo last.  Output y^T chunks drain via ScalarE/VectorE
    copies (alternating) and leave on two DMA queues.
"""

import numpy as np
import ml_dtypes

B, S, D = 4, 2048, 1024
H = 16
HH = 8          # heads per core
DK = 64
HD = 512        # head dims per core
N_CORES = 8
SCALE = DK ** -0.5
PB = 128        # partition block
QB = 512        # query block (matmul free dim)
NQB = S // QB   # 4
NKB = S // PB   # 16
KD = D // PB    # 8
KO = HD // PB   # 4
N_WARM = 14     # leading warm-up matmuls (HAM un-throttle)
N_FILL = 8      # trailing zero-dep PE filler matmuls

_COMPILED = None
LAST_RESULTS = None


def _build():
    from contextlib import ExitStack
    import concourse.bass as bass
    import concourse.tile as tile
    from concourse import bacc, mybir

    BF16 = mybir.dt.bfloat16
    F32 = mybir.dt.float32
    AF = mybir.ActivationFunctionType

    nc = bacc.Bacc("TRN2", target_bir_lowering=False, debug=False,
                   num_devices=N_CORES)

    xT_d = nc.dram_tensor("xT", [D, S], BF16, kind="ExternalInput")
    wq_d = nc.dram_tensor("wq", [D, HD], BF16, kind="ExternalInput")
    wk_d = nc.dram_tensor("wk", [D, HD], BF16, kind="ExternalInput")
    wv_d = nc.dram_tensor("wv", [D, HD], BF16, kind="ExternalInput")
    wo_d = nc.dram_tensor("wo", [HD, D], BF16, kind="ExternalInput")
    bq_d = nc.dram_tensor("bq", [PB, KO], F32, kind="ExternalInput")
    bk_d = nc.dram_tensor("bk", [PB, KO], F32, kind="ExternalInput")
    bvb_d = nc.dram_tensor("bvb", [PB, HD], BF16, kind="ExternalInput")
    mk_d = nc.dram_tensor("mk", [PB, 2 * PB], BF16, kind="ExternalInput")
    yT_d = nc.dram_tensor("yT", [D, S], F32, kind="ExternalOutput")

    with tile.TileContext(nc) as tc, ExitStack() as ctx:
        persist = ctx.enter_context(tc.tile_pool(name="persist", bufs=1))
        work = ctx.enter_context(tc.tile_pool(name="work", bufs=6))
        nrm = ctx.enter_context(tc.tile_pool(name="nrm", bufs=4))
        # PSUM: 8 banks total.  scores 2x2 + PV accumulators 2x1 +
        # projection accumulators 2x1 = 8.
        psS = ctx.enter_context(tc.tile_pool(name="psS", bufs=2, space="PSUM"))
        psO = ctx.enter_context(tc.tile_pool(name="psO", bufs=2, space="PSUM"))
        psP = ctx.enter_context(tc.tile_pool(name="psP", bufs=2, space="PSUM"))

        xT = [persist.tile([PB, S], BF16, name=f"xT{k}") for k in range(KD)]
        wq = [persist.tile([PB, HD], BF16, name=f"wq{k}") for k in range(KD)]
        wk = [persist.tile([PB, HD], BF16, name=f"wk{k}") for k in range(KD)]
        wv = [persist.tile([PB, HD], BF16, name=f"wv{k}") for k in range(KD)]
        wo = [persist.tile([PB, D], BF16, name=f"wo{k}") for k in range(KO)]
        bq = persist.tile([PB, KO], F32)
        bk = persist.tile([PB, KO], F32)
        bvb = persist.tile([PB, HD], BF16)
        mk = persist.tile([PB, 2, PB], BF16)
        qT = [persist.tile([PB, S], BF16, name=f"qT{k}") for k in range(KO)]
        kT = [persist.tile([PB, S], BF16, name=f"kT{k}") for k in range(KO)]
        # per key-block: 8 heads x (64 ones columns + 64 V columns).
        # Ones first so the PV matmul puts the softmax denominator on
        # partitions 0..63 (reciprocal_approx_fast requires base partition 0).
        v = [persist.tile([PB, HH, 2 * DK], BF16, name=f"v{k}") for k in range(NKB)]
        onorm = [persist.tile([PB, S], BF16, name=f"onorm{k}") for k in range(KO)]
        wsrc = persist.tile([PB, QB], BF16, name="wsrc")

        # --- warm-up: PE busy from t=0 so HAM un-throttles early ---------
        nc.vector.memset(wsrc[:], 0.0)

        def warm_mm():
            wps = psP.tile([PB, QB], F32, tag="p")
            nc.tensor.matmul(wps[:], wsrc[:, 0:PB], wsrc[:],
                             start=True, stop=True)

        for _ in range(N_WARM):
            warm_mm()

        # --- input DMA: small tensors first, V/Q/K weights interleaved ---
        nc.sync.dma_start(bq[:], bq_d[:])
        nc.sync.dma_start(bk[:], bk_d[:])
        nc.sync.dma_start(mk[:], mk_d[:].rearrange("p (j c) -> p j c", j=2))
        nc.gpsimd.dma_start(bvb[:], bvb_d[:])
        for k in range(KD):
            nc.gpsimd.dma_start(wv[k][:], wv_d[k * PB:(k + 1) * PB, :])
            nc.gpsimd.dma_start(wq[k][:], wq_d[k * PB:(k + 1) * PB, :])
            nc.gpsimd.dma_start(wk[k][:], wk_d[k * PB:(k + 1) * PB, :])
        SH = S // 2
        for k in range(KD):
            nc.sync.dma_start(xT[k][:, 0:SH], xT_d[k * PB:(k + 1) * PB, 0:SH])
        for k in range(KD):
            nc.sync.dma_start(xT[k][:, SH:S], xT_d[k * PB:(k + 1) * PB, SH:S])
        for k2 in range(KO):
            nc.gpsimd.dma_start(wo[k2][:], wo_d[k2 * PB:(k2 + 1) * PB, :])
        for kb in range(NKB):
            nc.vector.memset(v[kb][:, :, 0:DK], 1.0)

        # --- V projection, one key-block (128 seq positions) per chunk ---
        def v_proj(kb):
            accv = psP.tile([PB, QB], F32, tag="p")
            for k in range(KD):
                nc.tensor.matmul(accv[:], xT[k][:, kb * PB:(kb + 1) * PB],
                                 wv[k][:], start=(k == 0), stop=(k == KD - 1),
                                 skip_group_check=True)
            nc.vector.tensor_add(
                v[kb][:, :, DK:2 * DK],
                accv[:].rearrange("p (h d) -> p h d", h=HH),
                bvb[:].rearrange("p (h d) -> p h d", h=HH))

        def qk_proj(m, qb):
            qs = slice(qb * QB, (qb + 1) * QB)
            accq = psP.tile([PB, QB], F32, tag="p")
            for k in range(KD):
                nc.tensor.matmul(accq[:], wq[k][:, m * PB:(m + 1) * PB],
                                 xT[k][:, qs], start=(k == 0),
                                 stop=(k == KD - 1), skip_group_check=True)
            nc.vector.tensor_scalar_add(qT[m][:, qs], accq[:], bq[:, m:m + 1])
            acck = psP.tile([PB, QB], F32, tag="p")
            for k in range(KD):
                nc.tensor.matmul(acck[:], wk[k][:, m * PB:(m + 1) * PB],
                                 xT[k][:, qs], start=(k == 0),
                                 stop=(k == KD - 1), skip_group_check=True)
            nc.vector.tensor_scalar_add(kT[m][:, qs], acck[:], bk[:, m:m + 1])

        def attention(hp, qb):
            # head pair 2*hp (rows 0:64) + 2*hp+1 (rows 64:128), query block qb
            m = hp
            qs = slice(qb * QB, (qb + 1) * QB)
            nkb = 4 * qb + 4
            o_accs = [psO.tile([PB, QB], F32, tag="oacc", name=f"oacc{i}")
                      for i in range(2)]
            for kb in range(nkb):
                t = kb - 4 * qb
                c0 = 0 if t < 0 else PB * t
                cs = slice(qb * QB + c0, (qb + 1) * QB)
                s_ps = psS.tile([PB, 2, QB], F32, tag="s")
                for i, rb in enumerate((0, DK)):
                    nc.tensor.matmul(
                        s_ps[:, i, c0:QB],
                        kT[m][rb:rb + DK, kb * PB:(kb + 1) * PB],
                        qT[m][rb:rb + DK, cs], start=True, stop=True)
                e_sb = work.tile([PB, 2, QB], BF16, tag="exp")
                nc.scalar.activation(e_sb[:, :, c0:QB], s_ps[:, :, c0:QB],
                                     AF.Exp)
                if t >= 0:
                    nc.vector.tensor_mul(e_sb[:, :, c0:c0 + PB],
                                         e_sb[:, :, c0:c0 + PB], mk[:])
                for i in range(2):
                    h = 2 * hp + i
                    nc.tensor.matmul(o_accs[i][:, c0:QB],
                                     v[kb][:, h, :], e_sb[:, i, c0:QB],
                                     start=(kb == 0), stop=(kb == nkb - 1),
                                     skip_group_check=True)
            for i, rb in enumerate((0, DK)):
                r_sb = nrm.tile([DK, QB], F32, tag="r")
                nc.vector.reciprocal_approx_fast(r_sb[:], o_accs[i][0:DK, :])
                nc.vector.tensor_mul(onorm[m][rb:rb + DK, qs],
                                     o_accs[i][DK:2 * DK, :], r_sb[:])

        # V for the first diagonal band, Q/K for head-pair 0, rest of V.
        for kb in range(NKB // 2):
            v_proj(kb)
        for qb in range(NQB):
            qk_proj(0, qb)
        for kb in range(NKB // 2, NKB):
            v_proj(kb)

        # Attention hp-major; qk projection of the next head pair is emitted
        # after each block so the scheduler can fill exp-latency stalls.
        for hp in range(KO):
            for qb in range(NQB):
                attention(hp, qb)
                if hp + 1 < KO:
                    qk_proj(hp + 1, qb)

        # Out projection partial: Y^T = sum_k2 Wo[k2].T @ O_cat^T[k2].
        # Drains alternate ScalarE/VectorE; DMA alternates two queues.
        for n, (mo2, qb) in enumerate(
                (m2, q) for m2 in range(KD // 2) for q in range(NQB)):
            qs = slice(qb * QB, (qb + 1) * QB)
            y_ps = psS.tile([PB, 2, QB], F32, tag="s")
            for j in range(2):
                mo = 2 * mo2 + j
                for k2 in range(KO):
                    nc.tensor.matmul(y_ps[:, j, :],
                                     wo[k2][:, mo * PB:(mo + 1) * PB],
                                     onorm[k2][:, qs], start=(k2 == 0),
                                     stop=(k2 == KO - 1),
                                     skip_group_check=True)
            y_sb = nrm.tile([PB, 2, QB], F32, tag="y")
            if n % 2 == 0:
                nc.vector.tensor_copy(y_sb[:], y_ps[:])
            else:
                nc.scalar.copy(y_sb[:], y_ps[:])
            dma_eng = nc.sync if n % 2 == 0 else nc.gpsimd
            dma_eng.dma_start(
                yT_d[2 * mo2 * PB:(2 * mo2 + 2) * PB, qs]
                .rearrange("(j p) q -> p j q", j=2), y_sb[:])

        # trailing zero-dep PE filler (keeps HAM warm through any bubble)
        for _ in range(N_FILL):
            warm_mm()

    nc.compile()
    return nc


def _get_compiled():
    global _COMPILED
    if _COMPILED is None:
        _COMPILED = _build()
    return _COMPILED


def _make_in_maps(x, Wq, bq, Wk, bk, Wv, bv, Wo):
    bf16 = ml_dtypes.bfloat16
    f32 = np.float32

    # inclusive lower-triangular mask for diagonal 128x128 blocks
    p_idx = np.arange(PB)[:, None]
    c_idx = np.arange(PB)[None, :]
    mk = np.tile((p_idx <= c_idx).astype(bf16), (1, 2))

    in_maps = []
    for c in range(N_CORES):
        b, hh = c // 2, c % 2
        cs = slice(hh * HD, (hh + 1) * HD)
        in_maps.append({
            "xT": np.ascontiguousarray(x[b].T).astype(bf16),
            "wq": (Wq[:, cs] * SCALE).astype(bf16),
            "wk": np.ascontiguousarray(Wk[:, cs]).astype(bf16),
            "wv": np.ascontiguousarray(Wv[:, cs]).astype(bf16),
            "wo": np.ascontiguousarray(Wo[cs, :]).astype(bf16),
            "bq": np.ascontiguousarray(
                (bq[cs] * SCALE).astype(f32).reshape(KO, PB).T),
            "bk": np.ascontiguousarray(
                bk[cs].astype(f32).reshape(KO, PB).T),
            "bvb": np.broadcast_to(bv[cs].astype(bf16), (PB, HD)).copy(),
            "mk": mk,
        })
    return in_maps


def _reference_fallback(x, mask, Wq, bq, Wk, bk, Wv, bv, Wo, bo):
    out = np.empty((B, S, D), dtype=np.float32)
    for b in range(B):
        q = (x[b] @ Wq + bq).reshape(S, H, DK).transpose(1, 0, 2)
        k = (x[b] @ Wk + bk).reshape(S, H, DK).transpose(1, 0, 2)
        vv = (x[b] @ Wv + bv).reshape(S, H, DK).transpose(1, 0, 2)
        o = np.empty((H, S, DK), dtype=np.float32)
        for hi in range(H):
            s = (q[hi] @ k[hi].T) * SCALE
            s = np.where(mask[b], -1e9, s)
            s = s - s.max(axis=-1, keepdims=True)
            e = np.exp(s)
            p = e / e.sum(axis=-1, keepdims=True)
            o[hi] = p @ vv[hi]
        out[b] = o.transpose(1, 0, 2).reshape(S, D) @ Wo + bo
    return out


def kernel(x, mask, Wq, bq, Wk, bk, Wv, bv, Wo, bo, **kwargs):
    global LAST_RESULTS
    import os

    x = np.asarray(x, dtype=np.float32)
    mask = np.asarray(mask)

    causal = np.triu(np.ones((S, S), dtype=bool), k=1)
    if not all(np.array_equal(mask[b], causal) for b in range(B)):
        return _reference_fallback(np.asarray(x), mask, np.asarray(Wq),
                                   np.asarray(bq), np.asarray(Wk),
                                   np.asarray(bk), np.asarray(Wv),
                                   np.asarray(bv), np.asarray(Wo),
                                   np.asarray(bo))

    from concourse.bass_utils import run_bass_kernel_spmd

    nc = _get_compiled()
    in_maps = _make_in_maps(x, np.asarray(Wq), np.asarray(bq), np.asarray(Wk),
                            np.asarray(bk), np.asarray(Wv), np.asarray(bv),
                            np.asarray(Wo))
    trace = bool(int(os.environ.get("KERNEL_PROFILE", "0")))
    res = run_bass_kernel_spmd(nc, in_maps, list(range(N_CORES)), trace=trace)
    LAST_RESULTS = res

    bo32 = np.asarray(bo, dtype=np.float32)
    out = np.empty((B, S, D), dtype=np.float32)
    for b in range(B):
        acc = res.results[2 * b]["yT"] + res.results[2 * b + 1]["yT"]
        out[b] = acc.T + bo32
    return out


# revision 7
# speedup vs baseline: 1.0288x; 1.0137x over previous
"""Causal multi-head attention (B=4, S=2048, D=1024, H=16) on 8 Trainium2 cores.

Sharding: core c handles batch b = c//2 and head-half hh = c%2 (8 heads, 512
head-dims). QKV/out projections are tensor-parallel over the head dim;
attention is embarrassingly parallel over (b, head). The out-projection
partial products (rank-512 each) are summed pairwise on the host along with
the output bias.

On-device layout is fully transposed (d_model / head-dim on partitions,
sequence on the free axis) so every matmul contracts over the partition dim
with no on-chip transposes:
  Q^T = (Wq*scale)-tiles.T @ x^T      K^T likewise     V natural = x^T.T @ Wv
  S^T[k,q] = K_h @ Q_h^T              exp on ScalarE (no max subtraction:
                                      |scores| <~ 5, exp is safe in fp32)
  [l..l; O^T] = [1s | V_h].T @ expS^T (64 ones columns in the stationary make
                                      PV emit the softmax denominator
                                      replicated on partitions 0..63)
  O_norm^T = O^T * (1/l)              Y^T-partial = Wo-tiles.T @ O_cat^T
Causality: fully-masked key-blocks are skipped; diagonal blocks compute only
the valid column range and apply one 128x128 triangular mask multiply.
Head pairs (even/odd) interleave so K=64 score matmuls pack into disjoint
PE row-groups and run concurrently.

Schedule notes (v2):
  - ScalarE exp demand inside an attention block exceeds the PE demand, so
    the PE needs independent projection work to chew on while exp drains.
    Scores, PV accumulators and projection accumulators live in separate
    PSUM pools (4+2+2 banks) so projection matmuls are never blocked on a
    score buffer held by ScalarE.
  - ~14 warm-up matmuls on a zeroed tile run at t=0 so the PE HAM clock
    gate lifts (1.2 -> 2.4 GHz) during the DMA lead-in instead of ~35us in;
    a few more zero-dep matmuls at the tail of the program act as idle
    filler wherever the scheduler finds a PE bubble.
  - Input DMA order: biases/mask first, then V/Q/K weights interleaved
    per k-block on one queue while x^T streams (split in column halves)
    on the other; Wo last.  Output y^T chunks drain via ScalarE/VectorE
    copies (alternating) and leave on two DMA queues.
"""

import numpy as np
import ml_dtypes

B, S, D = 4, 2048, 1024
H = 16
HH = 8          # heads per core
DK = 64
HD = 512        # head dims per core
N_CORES = 8
SCALE = DK ** -0.5
PB = 128        # partition block
QB = 512        # query block (matmul free dim)
NQB = S // QB   # 4
NKB = S // PB   # 16
KD = D // PB    # 8
KO = HD // PB   # 4
N_WARM = 14     # leading warm-up matmuls (HAM un-throttle)
N_FILL = 32     # trailing zero-dep PE filler matmuls (lead-in/bubble fill)

_COMPILED = None
LAST_RESULTS = None


def _build():
    from contextlib import ExitStack
    import concourse.bass as bass
    import concourse.tile as tile
    from concourse import bacc, mybir

    BF16 = mybir.dt.bfloat16
    F32 = mybir.dt.float32
    AF = mybir.ActivationFunctionType

    nc = bacc.Bacc("TRN2", target_bir_lowering=False, debug=False,
                   num_devices=N_CORES)

    xT_d = nc.dram_tensor("xT", [D, S], BF16, kind="ExternalInput")
    wq_d = nc.dram_tensor("wq", [D, HD], BF16, kind="ExternalInput")
    wk_d = nc.dram_tensor("wk", [D, HD], BF16, kind="ExternalInput")
    wv_d = nc.dram_tensor("wv", [D, HD], BF16, kind="ExternalInput")
    wo_d = nc.dram_tensor("wo", [HD, D], BF16, kind="ExternalInput")
    bq_d = nc.dram_tensor("bq", [PB, KO], F32, kind="ExternalInput")
    bk_d = nc.dram_tensor("bk", [PB, KO], F32, kind="ExternalInput")
    bvb_d = nc.dram_tensor("bvb", [PB, HD], BF16, kind="ExternalInput")
    mk_d = nc.dram_tensor("mk", [PB, 2 * PB], BF16, kind="ExternalInput")
    yT_d = nc.dram_tensor("yT", [D, S], F32, kind="ExternalOutput")

    with tile.TileContext(nc) as tc, ExitStack() as ctx:
        persist = ctx.enter_context(tc.tile_pool(name="persist", bufs=1))
        work = ctx.enter_context(tc.tile_pool(name="work", bufs=6))
        nrm = ctx.enter_context(tc.tile_pool(name="nrm", bufs=4))
        # PSUM: 8 banks total.  scores 2x2 + PV accumulators 2x1 +
        # projection accumulators 2x1 = 8.
        psS = ctx.enter_context(tc.tile_pool(name="psS", bufs=2, space="PSUM"))
        psO = ctx.enter_context(tc.tile_pool(name="psO", bufs=2, space="PSUM"))
        psP = ctx.enter_context(tc.tile_pool(name="psP", bufs=2, space="PSUM"))

        xT = [persist.tile([PB, S], BF16, name=f"xT{k}") for k in range(KD)]
        wq = [persist.tile([PB, HD], BF16, name=f"wq{k}") for k in range(KD)]
        wk = [persist.tile([PB, HD], BF16, name=f"wk{k}") for k in range(KD)]
        wv = [persist.tile([PB, HD], BF16, name=f"wv{k}") for k in range(KD)]
        wo = [persist.tile([PB, D], BF16, name=f"wo{k}") for k in range(KO)]
        bq = persist.tile([PB, KO], F32)
        bk = persist.tile([PB, KO], F32)
        bvb = persist.tile([PB, HD], BF16)
        mk = persist.tile([PB, 2, PB], BF16)
        qT = [persist.tile([PB, S], BF16, name=f"qT{k}") for k in range(KO)]
        kT = [persist.tile([PB, S], BF16, name=f"kT{k}") for k in range(KO)]
        # per key-block: 8 heads x (64 ones columns + 64 V columns).
        # Ones first so the PV matmul puts the softmax denominator on
        # partitions 0..63 (reciprocal_approx_fast requires base partition 0).
        v = [persist.tile([PB, HH, 2 * DK], BF16, name=f"v{k}") for k in range(NKB)]
        onorm = [persist.tile([PB, S], BF16, name=f"onorm{k}") for k in range(KO)]
        wsrc = persist.tile([PB, QB], BF16, name="wsrc")

        # --- warm-up: PE busy from t=0 so HAM un-throttles early ---------
        nc.vector.memset(wsrc[:], 0.0)

        def warm_mm(cols=QB):
            wps = psP.tile([PB, QB], F32, tag="p")
            nc.tensor.matmul(wps[:, 0:cols], wsrc[:, 0:PB], wsrc[:, 0:cols],
                             start=True, stop=True)

        for _ in range(N_WARM):
            warm_mm()

        # --- input DMA: small tensors, then Q/K weights (attention-critical
        # path: scores->exp saturates ScalarE early), then V, then Wo.  xT
        # streams in query-block column chunks, k-minor, so qk_proj(0, qb)
        # and v_proj unlock progressively during the DMA phase.
        nc.sync.dma_start(bq[:], bq_d[:])
        nc.sync.dma_start(bk[:], bk_d[:])
        nc.sync.dma_start(mk[:], mk_d[:].rearrange("p (j c) -> p j c", j=2))
        nc.gpsimd.dma_start(bvb[:], bvb_d[:])
        for k in range(KD):
            nc.gpsimd.dma_start(wq[k][:], wq_d[k * PB:(k + 1) * PB, :])
            nc.gpsimd.dma_start(wk[k][:], wk_d[k * PB:(k + 1) * PB, :])
        for k in range(KD):
            nc.gpsimd.dma_start(wv[k][:], wv_d[k * PB:(k + 1) * PB, :])
        for q in range(NQB):
            qs = slice(q * QB, (q + 1) * QB)
            for k in range(KD):
                nc.sync.dma_start(xT[k][:, qs], xT_d[k * PB:(k + 1) * PB, qs])
        for k2 in range(KO):
            nc.gpsimd.dma_start(wo[k2][:], wo_d[k2 * PB:(k2 + 1) * PB, :])
        for kb in range(NKB):
            nc.vector.memset(v[kb][:, :, 0:DK], 1.0)

        # --- V projection, one key-block (128 seq positions) per chunk ---
        def v_proj(kb):
            accv = psP.tile([PB, QB], F32, tag="p")
            for k in range(KD):
                nc.tensor.matmul(accv[:], xT[k][:, kb * PB:(kb + 1) * PB],
                                 wv[k][:], start=(k == 0), stop=(k == KD - 1),
                                 skip_group_check=True)
            nc.vector.tensor_add(
                v[kb][:, :, DK:2 * DK],
                accv[:].rearrange("p (h d) -> p h d", h=HH),
                bvb[:].rearrange("p (h d) -> p h d", h=HH))

        def qk_proj(m, qb):
            qs = slice(qb * QB, (qb + 1) * QB)
            accq = psP.tile([PB, QB], F32, tag="p")
            for k in range(KD):
                nc.tensor.matmul(accq[:], wq[k][:, m * PB:(m + 1) * PB],
                                 xT[k][:, qs], start=(k == 0),
                                 stop=(k == KD - 1), skip_group_check=True)
            nc.vector.tensor_scalar_add(qT[m][:, qs], accq[:], bq[:, m:m + 1])
            acck = psP.tile([PB, QB], F32, tag="p")
            for k in range(KD):
                nc.tensor.matmul(acck[:], wk[k][:, m * PB:(m + 1) * PB],
                                 xT[k][:, qs], start=(k == 0),
                                 stop=(k == KD - 1), skip_group_check=True)
            nc.vector.tensor_scalar_add(kT[m][:, qs], acck[:], bk[:, m:m + 1])

        def attention(hp, qb):
            # head pair 2*hp (rows 0:64) + 2*hp+1 (rows 64:128), query block qb
            m = hp
            qs = slice(qb * QB, (qb + 1) * QB)
            nkb = 4 * qb + 4
            o_accs = [psO.tile([PB, QB], F32, tag="oacc", name=f"oacc{i}")
                      for i in range(2)]
            for kb in range(nkb):
                t = kb - 4 * qb
                c0 = 0 if t < 0 else PB * t
                cs = slice(qb * QB + c0, (qb + 1) * QB)
                s_ps = psS.tile([PB, 2, QB], F32, tag="s")
                for i, rb in enumerate((0, DK)):
                    nc.tensor.matmul(
                        s_ps[:, i, c0:QB],
                        kT[m][rb:rb + DK, kb * PB:(kb + 1) * PB],
                        qT[m][rb:rb + DK, cs], start=True, stop=True)
                e_sb = work.tile([PB, 2, QB], BF16, tag="exp")
                nc.scalar.activation(e_sb[:, :, c0:QB], s_ps[:, :, c0:QB],
                                     AF.Exp)
                if t >= 0:
                    nc.vector.tensor_mul(e_sb[:, :, c0:c0 + PB],
                                         e_sb[:, :, c0:c0 + PB], mk[:])
                for i in range(2):
                    h = 2 * hp + i
                    nc.tensor.matmul(o_accs[i][:, c0:QB],
                                     v[kb][:, h, :], e_sb[:, i, c0:QB],
                                     start=(kb == 0), stop=(kb == nkb - 1),
                                     skip_group_check=True)
            for i, rb in enumerate((0, DK)):
                r_sb = nrm.tile([DK, QB], F32, tag="r")
                nc.vector.reciprocal_approx_fast(r_sb[:], o_accs[i][0:DK, :])
                nc.vector.tensor_mul(onorm[m][rb:rb + DK, qs],
                                     o_accs[i][DK:2 * DK, :], r_sb[:])

        # hp=0 attention interleaved with the projections that unlock as xT
        # streams in; priority order favors the scores->exp critical path.
        qk_proj(0, 0)
        for kb in range(4):
            v_proj(kb)
        qk_proj(0, 1)
        attention(0, 0)
        for kb in range(4, 8):
            v_proj(kb)
        qk_proj(0, 2)
        attention(0, 1)
        for kb in range(8, 12):
            v_proj(kb)
        qk_proj(0, 3)
        attention(0, 2)
        for kb in range(12, 16):
            v_proj(kb)
        qk_proj(1, 0)
        attention(0, 3)
        for qb in range(1, NQB):
            qk_proj(1, qb)

        # Remaining head pairs; qk projection of the next pair is emitted
        # after each block so the scheduler can fill exp-latency stalls.
        # hp=3 runs qb descending: its qb=3 block completes first, making
        # out-projection chunks available as PE filler for the final blocks.
        for hp in range(1, KO):
            qbs = range(NQB) if hp < KO - 1 else range(NQB - 1, -1, -1)
            for qb in qbs:
                attention(hp, qb)
                if hp + 1 < KO:
                    qk_proj(hp + 1, qb)

        # Out projection partial: Y^T = sum_k2 Wo[k2].T @ O_cat^T[k2].
        # qb descending matches readiness under hp=3's descending order.
        # Drains alternate ScalarE/VectorE; DMA alternates two queues.
        for n, (qb, mo2) in enumerate(
                (q, m2) for q in range(NQB - 1, -1, -1)
                for m2 in range(KD // 2)):
            qs = slice(qb * QB, (qb + 1) * QB)
            y_ps = psS.tile([PB, 2, QB], F32, tag="s")
            for j in range(2):
                mo = 2 * mo2 + j
                for k2 in range(KO):
                    nc.tensor.matmul(y_ps[:, j, :],
                                     wo[k2][:, mo * PB:(mo + 1) * PB],
                                     onorm[k2][:, qs], start=(k2 == 0),
                                     stop=(k2 == KO - 1),
                                     skip_group_check=True)
            y_sb = nrm.tile([PB, 2, QB], F32, tag="y")
            if n % 2 == 0:
                nc.vector.tensor_copy(y_sb[:], y_ps[:])
            else:
                nc.scalar.copy(y_sb[:], y_ps[:])
            dma_eng = nc.sync if n % 2 == 0 else nc.gpsimd
            dma_eng.dma_start(
                yT_d[2 * mo2 * PB:(2 * mo2 + 2) * PB, qs]
                .rearrange("(j p) q -> p j q", j=2), y_sb[:])

        # trailing zero-dep PE filler (keeps HAM warm through any bubble)
        for _ in range(N_FILL):
            warm_mm(cols=PB * 2)

    nc.compile()
    return nc


def _get_compiled():
    global _COMPILED
    if _COMPILED is None:
        _COMPILED = _build()
    return _COMPILED


def _make_in_maps(x, Wq, bq, Wk, bk, Wv, bv, Wo):
    bf16 = ml_dtypes.bfloat16
    f32 = np.float32

    # inclusive lower-triangular mask for diagonal 128x128 blocks
    p_idx = np.arange(PB)[:, None]
    c_idx = np.arange(PB)[None, :]
    mk = np.tile((p_idx <= c_idx).astype(bf16), (1, 2))

    in_maps = []
    for c in range(N_CORES):
        b, hh = c // 2, c % 2
        cs = slice(hh * HD, (hh + 1) * HD)
        in_maps.append({
            "xT": np.ascontiguousarray(x[b].T).astype(bf16),
            "wq": (Wq[:, cs] * SCALE).astype(bf16),
            "wk": np.ascontiguousarray(Wk[:, cs]).astype(bf16),
            "wv": np.ascontiguousarray(Wv[:, cs]).astype(bf16),
            "wo": np.ascontiguousarray(Wo[cs, :]).astype(bf16),
            "bq": np.ascontiguousarray(
                (bq[cs] * SCALE).astype(f32).reshape(KO, PB).T),
            "bk": np.ascontiguousarray(
                bk[cs].astype(f32).reshape(KO, PB).T),
            "bvb": np.broadcast_to(bv[cs].astype(bf16), (PB, HD)).copy(),
            "mk": mk,
        })
    return in_maps


def _reference_fallback(x, mask, Wq, bq, Wk, bk, Wv, bv, Wo, bo):
    out = np.empty((B, S, D), dtype=np.float32)
    for b in range(B):
        q = (x[b] @ Wq + bq).reshape(S, H, DK).transpose(1, 0, 2)
        k = (x[b] @ Wk + bk).reshape(S, H, DK).transpose(1, 0, 2)
        vv = (x[b] @ Wv + bv).reshape(S, H, DK).transpose(1, 0, 2)
        o = np.empty((H, S, DK), dtype=np.float32)
        for hi in range(H):
            s = (q[hi] @ k[hi].T) * SCALE
            s = np.where(mask[b], -1e9, s)
            s = s - s.max(axis=-1, keepdims=True)
            e = np.exp(s)
            p = e / e.sum(axis=-1, keepdims=True)
            o[hi] = p @ vv[hi]
        out[b] = o.transpose(1, 0, 2).reshape(S, D) @ Wo + bo
    return out


def kernel(x, mask, Wq, bq, Wk, bk, Wv, bv, Wo, bo, **kwargs):
    global LAST_RESULTS
    import os

    x = np.asarray(x, dtype=np.float32)
    mask = np.asarray(mask)

    causal = np.triu(np.ones((S, S), dtype=bool), k=1)
    if not all(np.array_equal(mask[b], causal) for b in range(B)):
        return _reference_fallback(np.asarray(x), mask, np.asarray(Wq),
                                   np.asarray(bq), np.asarray(Wk),
                                   np.asarray(bk), np.asarray(Wv),
                                   np.asarray(bv), np.asarray(Wo),
                                   np.asarray(bo))

    from concourse.bass_utils import run_bass_kernel_spmd

    nc = _get_compiled()
    in_maps = _make_in_maps(x, np.asarray(Wq), np.asarray(bq), np.asarray(Wk),
                            np.asarray(bk), np.asarray(Wv), np.asarray(bv),
                            np.asarray(Wo))
    trace = bool(int(os.environ.get("KERNEL_PROFILE", "0")))
    res = run_bass_kernel_spmd(nc, in_maps, list(range(N_CORES)), trace=trace)
    LAST_RESULTS = res

    bo32 = np.asarray(bo, dtype=np.float32)
    out = np.empty((B, S, D), dtype=np.float32)
    for b in range(B):
        acc = res.results[2 * b]["yT"] + res.results[2 * b + 1]["yT"]
        out[b] = acc.T + bo32
    return out
